# revision 3
# baseline (speedup 1.0000x reference)
"""Trainium2 Bass kernel for a custom LSTM cell step.

Reference computation (per full problem, B=8192, D=U=512):
    z = inputs @ kernel + h_tm1 @ recurrent_kernel + bias        # [B, 4U]
    i, f, g, o = split(z, 4, axis=1)
    i, f, o = sigmoid(...)  ;  g = tanh(g)
    c = f * c_tm1 + i * g
    h = o * tanh(c)
    return (h, h, c)

Sharding: data-parallel over the batch dim across 8 NeuronCores
(1024 rows per core); weights replicated.

Kernel strategy (v2, fp8 DoubleRow):
  - The PE is the bottleneck: at 1.0 cyc/row (f32r / bf16) the 256
    gate-chunk matmuls per core cost 131072 rows ~= 54.6us at 2.4 GHz.
    fp8 (e4m3) with MatmulPerfMode.DoubleRow runs 0.5 cyc/row (2 K-rows
    per cycle, 256 effective K per instruction).
  - Precision: pure fp8 fails the 2e-2 gate (~3.8e-2). Instead use a
    hi/lo decomposition: a ~= a_hi + a_lo with both parts e4m3, operands
    pre-scaled by 16 on the host (lifts residuals out of the subnormal
    floor; descale 1/256 is fused into the ACT activation's scale).
      z*256 = xh_hi@W_hi + xh_lo@W_hi + xh_hi@W_lo   (lo@lo dropped)
    Measured end-to-end rel err ~1.3e-3 (vs 1.5e-4 for the f32r
    baseline, gate is 2e-2). PE cost: 3 terms * K=1024 at 0.5 cyc/row =
    98304 cycles ~= 41us.
  - x/h are pre-transposed AND pre-quantized on the host: no PE
    transposes, no PSUM->SBUF copies, no identity constant.
  - Emission pairs two gate chunks per stationary operand (t-step outer,
    2 gates inner) so each LDWEIGHTS is followed by 2x256 cycles of
    streaming, keeping the weight-load port off the critical path.
  - DMA issue is spread over 4 engines (sync: activations+c, vector and
    gpsimd: W_hi slabs, scalar: W_lo) because each DMA_DIRECT2D issue
    occupies its queueing engine ~0.65us; the first matmul only needs
    xh_hi(mt0-3) + W_hi[kg0:2, gate-i] ~= 640KB, so the PE starts ~2us
    after the DMA stream begins.
  - Phase A computes i and g for all 8 m-tiles (ig = i*g kept in SBUF),
    phase B computes f and o, then c/h, streaming results out per tile.
"""

from contextlib import ExitStack

import ml_dtypes
import numpy as np

import concourse.bass as bass  # noqa: F401  (kept for parity with docs)
import concourse.mybir as mybir
import concourse.tile as tile
from concourse import bacc
from concourse.bass_utils import run_bass_kernel_spmd

# Problem sizes (hardcoded per spec).
B, D, U = 8192, 512, 512
N_CORES = 8
MB = B // N_CORES  # 1024 batch rows per core
P = 128
MT = MB // P  # 8 m-tiles per core
KK = D + U  # 1024 stacked contraction dim (x|h vs W|R)
KG = KK // P  # 8 k-groups of 128
NG = 4 * U  # 2048 gate columns
NC = 512  # gate chunk width (one gate)

S = 16.0  # host-side operand pre-scale (per operand)
SINV = 1.0 / (S * S)  # fused descale in the activation

F32 = mybir.dt.float32
FP8 = mybir.dt.float8e4
E4NP = ml_dtypes.float8_e4m3

SIG = mybir.ActivationFunctionType.Sigmoid
TANH = mybir.ActivationFunctionType.Tanh
DR = mybir.MatmulPerfMode.DoubleRow

_NC_CACHE: dict = {}


def _build_lstm_nc(with_bias: bool):
    """Build and compile the per-core Bass program."""
    nc = bacc.Bacc("TRN2", target_bir_lowering=False, debug=False)

    xhi_d = nc.dram_tensor("xhi_t", [KK, MB], FP8, kind="ExternalInput")
    xlo_d = nc.dram_tensor("xlo_t", [KK, MB], FP8, kind="ExternalInput")
    whi_d = nc.dram_tensor("w_hi", [KK, NG], FP8, kind="ExternalInput")
    wlo_d = nc.dram_tensor("w_lo", [KK, NG], FP8, kind="ExternalInput")
    c_d = nc.dram_tensor("c_tm1", [MB, U], F32, kind="ExternalInput")
    b_d = None
    if with_bias:
        b_d = nc.dram_tensor("bias", [NG], F32, kind="ExternalInput")
    ho_d = nc.dram_tensor("h_out", [MB, U], F32, kind="ExternalOutput")
    co_d = nc.dram_tensor("c_out", [MB, U], F32, kind="ExternalOutput")

    # DRAM views tiled to [partition, group, free]
    xhi_v = xhi_d.ap().rearrange("(kg p) m -> p kg m", p=P)
    xlo_v = xlo_d.ap().rearrange("(kg p) m -> p kg m", p=P)
    whi_v = whi_d.ap().rearrange("(kg p) n -> p kg n", p=P)
    wlo_v = wlo_d.ap().rearrange("(kg p) n -> p kg n", p=P)
    c_v = c_d.ap().rearrange("(mt p) u -> p mt u", p=P)
    ho_v = ho_d.ap().rearrange("(mt p) u -> p mt u", p=P)
    co_v = co_d.ap().rearrange("(mt p) u -> p mt u", p=P)

    with tile.TileContext(nc) as tc, ExitStack() as ctx:
        consts = ctx.enter_context(tc.tile_pool(name="consts", bufs=1))
        igpool = ctx.enter_context(tc.tile_pool(name="igpool", bufs=MT))
        scratch = ctx.enter_context(tc.tile_pool(name="scratch", bufs=4))
        outp = ctx.enter_context(tc.tile_pool(name="outp", bufs=4))
        zpsum = ctx.enter_context(tc.tile_pool(name="zpsum", bufs=8, space="PSUM"))

        xhi_sb = consts.tile([P, KG, MB], FP8)
        xlo_sb = consts.tile([P, KG, MB], FP8)
        whi_sb = consts.tile([P, KG, NG], FP8)
        wlo_sb = consts.tile([P, KG, NG], FP8)
        c_sb = consts.tile([P, MT, U], F32)

        # --- DMA schedule.  Issues are spread across engines; within an
        # engine the order is the arrival order.  Gate columns: chunk 0=i,
        # 1=f, 2=g, 3=o.  Phase A consumes chunks 0,2; phase B chunks 1,3.
        HM = MB // 2  # half the m columns (4 m-tiles)

        # sync: activations halves, then c
        nc.sync.dma_start(xhi_sb[:, :, 0:HM], xhi_v[:, :, 0:HM])
        nc.sync.dma_start(xlo_sb[:, :, 0:HM], xlo_v[:, :, 0:HM])
        nc.sync.dma_start(xhi_sb[:, :, HM:MB], xhi_v[:, :, HM:MB])
        nc.sync.dma_start(xlo_sb[:, :, HM:MB], xlo_v[:, :, HM:MB])
        nc.sync.dma_start(c_sb[:], c_v)

        # gpsimd: W_hi gate-i in kg-pair slabs (paces the PE start), then
        # W_hi gate-f, gate-o for phase B, then the output DMAs below.
        for t in range(4):
            sl = slice(2 * t, 2 * t + 2)
            nc.gpsimd.dma_start(whi_sb[:, sl, 0:NC], whi_v[:, sl, 0:NC])
        nc.gpsimd.dma_start(whi_sb[:, :, NC : 2 * NC], whi_v[:, :, NC : 2 * NC])
        nc.gpsimd.dma_start(whi_sb[:, :, 3 * NC : NG], whi_v[:, :, 3 * NC : NG])

        # scalar: W_hi gate-g slabs (parallel with gpsimd's gate-i slabs,
        # phase A consumes both at the same t-step), then W_lo chunks in
        # first-use order (i, g, f, o).
        for t in range(4):
            sl = slice(2 * t, 2 * t + 2)
            nc.scalar.dma_start(
                whi_sb[:, sl, 2 * NC : 3 * NC], whi_v[:, sl, 2 * NC : 3 * NC]
            )
        nc.scalar.dma_start(wlo_sb[:, :, 0:NC], wlo_v[:, :, 0:NC])
        nc.scalar.dma_start(wlo_sb[:, :, 2 * NC : 3 * NC], wlo_v[:, :, 2 * NC : 3 * NC])
        nc.scalar.dma_start(wlo_sb[:, :, NC : 2 * NC], wlo_v[:, :, NC : 2 * NC])
        nc.scalar.dma_start(wlo_sb[:, :, 3 * NC : NG], wlo_v[:, :, 3 * NC : NG])

        bias_bc = None
        if with_bias:
            assert b_d is not None
            # bias arrives host-pre-scaled by 256 to match the scaled z.
            bias_bc = consts.tile([P, NG], F32)
            b_ap = b_d.ap()
            nc.gpsimd.dma_start(
                out=bias_bc,
                in_=bass.AP(tensor=b_ap.tensor, offset=b_ap.offset, ap=[[0, P], [1, NG]]),
            )

        def z_pair(n1, n2, mt):
            """Accumulate gate chunks n1 and n2 for m-tile mt.

            t-step outer, two gate streams inner: each stationary
            (lhsT) load feeds 2x512 output rows of streaming.
            """
            zp1 = zpsum.tile([P, NC], F32, tag="z")
            zp2 = zpsum.tile([P, NC], F32, tag="z")
            steps = (
                [(xhi_sb, whi_sb, t) for t in range(4)]
                + [(xlo_sb, whi_sb, t) for t in range(4)]
                + [(xhi_sb, wlo_sb, t) for t in range(4)]
            )
            for si, (xsb, wsb, t) in enumerate(steps):
                st, sp = si == 0, si == len(steps) - 1
                lhsT = xsb[:, 2 * t : 2 * t + 2, mt * P : (mt + 1) * P]
                for zp, n in ((zp1, n1), (zp2, n2)):
                    nc.tensor.matmul(
                        zp[:],
                        lhsT,
                        wsb[:, 2 * t : 2 * t + 2, n * NC : (n + 1) * NC],
                        start=st,
                        stop=sp,
                        perf_mode=DR,
                    )
            if bias_bc is not None:
                nc.vector.tensor_add(zp1[:], zp1[:], bias_bc[:, n1 * NC : (n1 + 1) * NC])
                nc.vector.tensor_add(zp2[:], zp2[:], bias_bc[:, n2 * NC : (n2 + 1) * NC])
            return zp1, zp2

        ig_t = {}

        def phase_a(mt):  # i = sig(z0), g = tanh(z2), ig = i*g
            zi, zg = z_pair(0, 2, mt)
            it = scratch.tile([P, NC], F32, tag="act")
            nc.scalar.activation(it[:], zi[:], SIG, scale=SINV)
            gt = scratch.tile([P, NC], F32, tag="act")
            nc.scalar.activation(gt[:], zg[:], TANH, scale=SINV)
            ig = igpool.tile([P, NC], F32, tag="ig")
            nc.vector.tensor_mul(ig[:], it[:], gt[:])
            ig_t[mt] = ig

        def phase_b(mt):  # f = sig(z1), o = sig(z3); c, h; stream out
            zf, zo = z_pair(1, 3, mt)
            ft = scratch.tile([P, NC], F32, tag="act")
            nc.scalar.activation(ft[:], zf[:], SIG, scale=SINV)
            ot = scratch.tile([P, NC], F32, tag="act")
            nc.scalar.activation(ot[:], zo[:], SIG, scale=SINV)
            c_new = outp.tile([P, NC], F32, tag="cnew")
            nc.vector.tensor_mul(c_new[:], ft[:], c_sb[:, mt, :])
            nc.vector.tensor_add(c_new[:], c_new[:], ig_t.pop(mt)[:])
            nc.gpsimd.dma_start(co_v[:, mt, :], c_new[:])
            th = scratch.tile([P, NC], F32, tag="act")
            nc.scalar.activation(th[:], c_new[:], TANH)
            h_new = outp.tile([P, NC], F32, tag="hnew")
            nc.vector.tensor_mul(h_new[:], ot[:], th[:])
            nc.gpsimd.dma_start(ho_v[:, mt, :], h_new[:])

        for mt in range(MT):
            phase_a(mt)
        for mt in range(MT):
            phase_b(mt)

    nc.compile()
    return nc


def _get_nc(with_bias: bool):
    if with_bias not in _NC_CACHE:
        _NC_CACHE[with_bias] = _build_lstm_nc(with_bias)
    return _NC_CACHE[with_bias]


def _prep(inputs, h_tm1, c_tm1, kernel, recurrent_kernel, bias):
    """Host-side quantization/transpose + per-core input maps."""
    x = np.asarray(inputs, dtype=np.float32)
    h = np.asarray(h_tm1, dtype=np.float32)
    c = np.ascontiguousarray(np.asarray(c_tm1, dtype=np.float32))
    w = np.asarray(kernel, dtype=np.float32)
    r = np.asarray(recurrent_kernel, dtype=np.float32)
    b = np.asarray(bias, dtype=np.float32)

    # Stacked, scaled, transposed activations: [KK, B]
    xh_t = np.empty((KK, B), dtype=np.float32)
    np.multiply(x.T, S, out=xh_t[:D])
    np.multiply(h.T, S, out=xh_t[D:])
    xhi_t = xh_t.astype(E4NP)
    xlo_t = (xh_t - xhi_t.astype(np.float32)).astype(E4NP)

    # Stacked, scaled weights: [KK, NG]
    wr = np.empty((KK, NG), dtype=np.float32)
    np.multiply(w, S, out=wr[:D])
    np.multiply(r, S, out=wr[D:])
    whi = wr.astype(E4NP)
    wlo = (wr - whi.astype(np.float32)).astype(E4NP)

    with_bias = bool(np.any(b))
    in_maps = []
    for core in range(N_CORES):
        sl = slice(core * MB, (core + 1) * MB)
        m = {
            "xhi_t": np.ascontiguousarray(xhi_t[:, sl]),
            "xlo_t": np.ascontiguousarray(xlo_t[:, sl]),
            "w_hi": whi,
            "w_lo": wlo,
            "c_tm1": np.ascontiguousarray(c[sl]),
        }
        if with_bias:
            m["bias"] = b * (S * S)
        in_maps.append(m)
    return in_maps, with_bias


def kernel(inputs, h_tm1, c_tm1, kernel, recurrent_kernel, bias):
    in_maps, with_bias = _prep(inputs, h_tm1, c_tm1, kernel, recurrent_kernel, bias)
    nc = _get_nc(with_bias)
    res = run_bass_kernel_spmd(nc, in_maps, core_ids=list(range(N_CORES)))
    h_out = np.concatenate([r_["h_out"] for r_ in res.results], axis=0)
    c_out = np.concatenate([r_["c_out"] for r_ in res.results], axis=0)
    return (h_out, h_out, c_out)


# revision 8
# speedup vs baseline: 1.3993x; 1.3993x over previous
"""Trainium2 Bass kernel for a custom LSTM cell step.

Reference computation (per full problem, B=8192, D=U=512):
    z = inputs @ kernel + h_tm1 @ recurrent_kernel + bias        # [B, 4U]
    i, f, g, o = split(z, 4, axis=1)
    i, f, o = sigmoid(...)  ;  g = tanh(g)
    c = f * c_tm1 + i * g
    h = o * tanh(c)
    return (h, h, c)

Sharding: data-parallel over the batch dim across 8 NeuronCores
(1024 rows per core); weights replicated.

Kernel strategy (v3, bf16 + host pre-transpose):
  - The PE is the bottleneck. Measured on hw: one [K=128]x[128,512]
    matmul streams at ~216ns regardless of f32r/bf16; fp8 DoubleRow
    doubles K per instruction but the 3-term hi/lo decomposition needed
    for precision costs 1.5x bf16's cycles, so bf16 is optimal:
    256 matmuls x 512 cycles ~= 55us/core at 2.4 GHz.
  - x/h are cast to bf16 AND pre-transposed on the host into a stacked
    [x^T; h^T] tensor: no PE transposes, no PSUM->SBUF copies. W/R are
    host-stacked [W; R] in bf16 (halves all weight DMA).
    bf16 quantization end-to-end rel err ~2.4e-3 (gate is 2e-2).
  - DMA: few, large descriptors with 2KB runs (small-run DMAs have
    multi-us issue costs on the queueing engine). xh arrives in
    kg-pair slabs (full m) so the i-phase can start ~2us after the DMA
    stream begins; W arrives as [kg-pair, half-N] slabs; gate phases
    are ordered i, f, g, o so phases 1+2 need only the first W half.
  - i-phase runs kg-outer/mt-inner across all 8 PSUM banks (paced by
    the arriving xh slabs); f/g/o phases run mt-outer/kg-inner.
  - ACT activations, DVE elementwise, and output DMAs trail the PE by
    a fraction of a phase; the tail after the last matmul is just
    sigmoid(o[mt7]) + h-mul + one 256KB DMA.
"""

from contextlib import ExitStack

import ml_dtypes
import numpy as np

import concourse.bass as bass
import concourse.mybir as mybir
import concourse.tile as tile
from concourse import bacc
from concourse.bass_utils import run_bass_kernel_spmd

# Problem sizes (hardcoded per spec).
B, D, U = 8192, 512, 512
N_CORES = 8
MB = B // N_CORES  # 1024 batch rows per core
P = 128
MT = MB // P  # 8 m-tiles per core
KK = D + U  # 1024 stacked contraction dim (x|h vs W|R)
KG = KK // P  # 8 k-groups of 128
NG = 4 * U  # 2048 gate columns
NC = 512  # gate chunk width (one gate)

F32 = mybir.dt.float32
BF16 = mybir.dt.bfloat16
BF16NP = ml_dtypes.bfloat16

SIG = mybir.ActivationFunctionType.Sigmoid
TANH = mybir.ActivationFunctionType.Tanh

# Gate column chunks: 0=i, 1=f, 2=g, 3=o
GI, GF, GG, GO = 0, 1, 2, 3

_NC_CACHE: dict = {}


def _build_lstm_nc(with_bias: bool):
    """Build and compile the per-core Bass program."""
    nc = bacc.Bacc("TRN2", target_bir_lowering=False, debug=False)

    xh_d = nc.dram_tensor("xh_t", [KK, MB], BF16, kind="ExternalInput")
    wr_d = nc.dram_tensor("wr", [KK, NG], BF16, kind="ExternalInput")
    c_d = nc.dram_tensor("c_tm1", [MB, U], F32, kind="ExternalInput")
    b_d = None
    if with_bias:
        b_d = nc.dram_tensor("bias", [NG], F32, kind="ExternalInput")
    ho_d = nc.dram_tensor("h_out", [MB, U], F32, kind="ExternalOutput")
    co_d = nc.dram_tensor("c_out", [MB, U], F32, kind="ExternalOutput")

    # DRAM views tiled to [partition, group, free]
    xh_v = xh_d.ap().rearrange("(kg p) m -> p kg m", p=P)
    wr_v = wr_d.ap().rearrange("(kg p) n -> p kg n", p=P)
    c_v = c_d.ap().rearrange("(mt p) u -> p mt u", p=P)
    ho_v = ho_d.ap().rearrange("(mt p) u -> p mt u", p=P)
    co_v = co_d.ap().rearrange("(mt p) u -> p mt u", p=P)

    with tile.TileContext(nc) as tc, ExitStack() as ctx:
        consts = ctx.enter_context(tc.tile_pool(name="consts", bufs=1))
        ipool = ctx.enter_context(tc.tile_pool(name="ipool", bufs=MT))
        fpool = ctx.enter_context(tc.tile_pool(name="fpool", bufs=MT))
        thpool = ctx.enter_context(tc.tile_pool(name="thpool", bufs=MT))
        scratch = ctx.enter_context(tc.tile_pool(name="scratch", bufs=4))
        outp = ctx.enter_context(tc.tile_pool(name="outp", bufs=4))
        zpsum = ctx.enter_context(tc.tile_pool(name="zpsum", bufs=8, space="PSUM"))

        xh_sb = consts.tile([P, KG, MB], BF16)
        wr_sb = consts.tile([P, KG, NG], BF16)
        c_sb = consts.tile([P, MT, U], F32)

        # Warm-up operands: a zero tile the PE can multiply while the
        # first input slabs are still in flight (spends the slow pstate
        # window on junk work so real matmuls start near full clock).
        junk = consts.tile([P, P + NC], BF16)
        nc.gpsimd.memset(junk[:], 0.0)
        jpsum = zpsum.tile([P, NC], F32, tag="z", name="junkbank")
        for _ in range(6):
            nc.tensor.matmul(
                jpsum[:], junk[:, 0:P], junk[:, P : P + NC], start=True, stop=True
            )

        # --- DMA schedule.  All transfers have 1-2KB contiguous runs.
        # The queues share fabric bandwidth fairly, so the slabs needed
        # first are kept small and the g/o weight queue is held back
        # behind a dummy read of the first xh slab.
        # sync: xh single-kg slabs (256KB each; pace the i-phase), then c.
        for kg in range(KG):
            sl = slice(kg, kg + 1)
            nc.sync.dma_start(xh_sb[:, sl, :], xh_v[:, sl, :])
        nc.sync.dma_start(c_sb[:], c_v)

        # scalar: W gate-i in single-kg slabs (128KB), then gate-f in
        # kg-pair slabs.
        for kg in range(KG):
            sl = slice(kg, kg + 1)
            nc.scalar.dma_start(wr_sb[:, sl, 0:NC], wr_v[:, sl, 0:NC])
        for t in range(4):
            sl = slice(2 * t, 2 * t + 2)
            nc.scalar.dma_start(wr_sb[:, sl, NC : 2 * NC], wr_v[:, sl, NC : 2 * NC])

        # gpsimd: wait until the first xh slab has landed (keeps these
        # 1MB transfers out of the prologue's bandwidth), then W gates
        # g,o in kg-pair slabs, then the h output DMAs below.
        gate_tile = consts.tile([P, 4], BF16)
        nc.gpsimd.tensor_copy(gate_tile[:], xh_sb[:, 0, 0:4])
        for t in range(4):
            sl = slice(2 * t, 2 * t + 2)
            nc.gpsimd.dma_start(wr_sb[:, sl, NG // 2 : NG], wr_v[:, sl, NG // 2 : NG])

        bias_bc = None
        if with_bias:
            assert b_d is not None
            bias_bc = consts.tile([P, NG], F32)
            b_ap = b_d.ap()
            nc.gpsimd.dma_start(
                out=bias_bc,
                in_=bass.AP(tensor=b_ap.tensor, offset=b_ap.offset, ap=[[0, P], [1, NG]]),
            )

        def add_bias(zp, n):
            if bias_bc is not None:
                nc.vector.tensor_add(zp[:], zp[:], bias_bc[:, n * NC : (n + 1) * NC])

        def z_chunk(n, mt):
            """Accumulate gate chunk n for m-tile mt (kg-inner)."""
            zp = zpsum.tile([P, NC], F32, tag="z")
            for kg in range(KG):
                nc.tensor.matmul(
                    zp[:],
                    xh_sb[:, kg, mt * P : (mt + 1) * P],
                    wr_sb[:, kg, n * NC : (n + 1) * NC],
                    start=(kg == 0),
                    stop=(kg == KG - 1),
                )
            add_bias(zp, n)
            return zp

        # --- i-phase: kg-outer / mt-inner across 8 PSUM banks, paced by
        # the xh kg-pair arrivals.
        i_banks = [
            zpsum.tile([P, NC], F32, tag="z", name=f"zbank{mt}") for mt in range(MT)
        ]
        for kg in range(KG):
            for mt in range(MT):
                nc.tensor.matmul(
                    i_banks[mt][:],
                    xh_sb[:, kg, mt * P : (mt + 1) * P],
                    wr_sb[:, kg, GI * NC : (GI + 1) * NC],
                    start=(kg == 0),
                    stop=(kg == KG - 1),
                )
        i_t = {}
        for mt in range(MT):
            add_bias(i_banks[mt], GI)
            it = ipool.tile([P, NC], F32, tag="i")
            nc.scalar.activation(it[:], i_banks[mt][:], SIG)
            i_t[mt] = it

        # --- f-phase: f = sigmoid(z1), kept for the g-phase.
        f_t = {}
        for mt in range(MT):
            zf = z_chunk(GF, mt)
            ft = fpool.tile([P, NC], F32, tag="f")
            nc.scalar.activation(ft[:], zf[:], SIG)
            f_t[mt] = ft

        # --- g-phase: g = tanh(z2); c = f*c_old + i*g; th = tanh(c).
        th_t = {}
        for mt in range(MT):
            zg = z_chunk(GG, mt)
            gt = scratch.tile([P, NC], F32, tag="act")
            nc.scalar.activation(gt[:], zg[:], TANH)
            ig = scratch.tile([P, NC], F32, tag="act")
            nc.vector.tensor_mul(ig[:], i_t.pop(mt)[:], gt[:])
            c_new = outp.tile([P, NC], F32, tag="cnew")
            nc.vector.tensor_mul(c_new[:], f_t.pop(mt)[:], c_sb[:, mt, :])
            nc.vector.tensor_add(c_new[:], c_new[:], ig[:])
            nc.sync.dma_start(co_v[:, mt, :], c_new[:])
            th = thpool.tile([P, NC], F32, tag="th")
            nc.scalar.activation(th[:], c_new[:], TANH)
            th_t[mt] = th

        # --- o-phase: o = sigmoid(z3); h = o * tanh(c).
        for mt in range(MT):
            zo = z_chunk(GO, mt)
            ot = scratch.tile([P, NC], F32, tag="act")
            nc.scalar.activation(ot[:], zo[:], SIG)
            h_new = outp.tile([P, NC], F32, tag="hnew")
            nc.vector.tensor_mul(h_new[:], ot[:], th_t.pop(mt)[:])
            nc.gpsimd.dma_start(ho_v[:, mt, :], h_new[:])

    nc.compile()
    return nc


def _get_nc(with_bias: bool):
    if with_bias not in _NC_CACHE:
        _NC_CACHE[with_bias] = _build_lstm_nc(with_bias)
    return _NC_CACHE[with_bias]


def _prep(inputs, h_tm1, c_tm1, kernel, recurrent_kernel, bias):
    """Host-side bf16 cast/transpose + per-core input maps."""
    x = np.asarray(inputs, dtype=np.float32)
    h = np.asarray(h_tm1, dtype=np.float32)
    c = np.ascontiguousarray(np.asarray(c_tm1, dtype=np.float32))
    w = np.asarray(kernel, dtype=np.float32)
    r = np.asarray(recurrent_kernel, dtype=np.float32)
    b = np.asarray(bias, dtype=np.float32)

    # Stacked transposed activations [KK, B] in bf16.
    xh_t = np.empty((KK, B), dtype=BF16NP)
    xh_t[:D] = x.T
    xh_t[D:] = h.T

    # Stacked weights [KK, NG] in bf16.
    wr = np.empty((KK, NG), dtype=BF16NP)
    wr[:D] = w
    wr[D:] = r

    with_bias = bool(np.any(b))
    in_maps = []
    for core in range(N_CORES):
        sl = slice(core * MB, (core + 1) * MB)
        m = {
            "xh_t": np.ascontiguousarray(xh_t[:, sl]),
            "wr": wr,
            "c_tm1": np.ascontiguousarray(c[sl]),
        }
        if with_bias:
            m["bias"] = b
        in_maps.append(m)
    return in_maps, with_bias


def kernel(inputs, h_tm1, c_tm1, kernel, recurrent_kernel, bias):
    in_maps, with_bias = _prep(inputs, h_tm1, c_tm1, kernel, recurrent_kernel, bias)
    nc = _get_nc(with_bias)
    for _attempt in range(3):
        res = run_bass_kernel_spmd(nc, in_maps, core_ids=list(range(N_CORES)))
        h_out = np.concatenate([r_["h_out"] for r_ in res.results], axis=0)
        c_out = np.concatenate([r_["c_out"] for r_ in res.results], axis=0)
        # Very first execution after device bring-up has been seen to
        # return garbage when a previous session's teardown overlaps;
        # a clean rerun is cheap insurance.
        if np.isfinite(h_out).all() and np.isfinite(c_out).all():
            break
    return (h_out, h_out, c_out)
